# revision 1
# baseline (speedup 1.0000x reference)
"""GQA forward (B=2,N=2048,D=2048,H=32,KV=8,DH=64, causal) on 8 trn2 cores.

Sharding: 2-way data parallel over batch x 4-way tensor parallel over heads
(each core: 8 q-heads = 2 kv-heads, keeping group structure). Row-parallel
out-proj; the all-reduce over the 4 TP shards (+ bias) happens on host at
gather time.

Device kernel (per core), all PE matmuls in float32r:
  phase 1: streaming projections from xT (host-pretransposed):
           Q^T (4 slabs of 2 heads), K^T, V^T -> V (PE transpose) augmented
           with a ones column (gives softmax row-sums for free in ctx matmul)
  phase 2: causal attention per head in S^T orientation:
           S^T = K^T.T @ Q^T per 128-key block (skipping above-diagonal
           blocks), exp on ACT with folded 1/sqrt(dh) scale, triangle mask
           multiply on diagonal blocks only, ctx^T accumulated in PSUM with
           row 64 = softmax denominator; normalize on the PSUM->SBUF copy.
  phase 3: out = ctx @ Wo_shard accumulated over 4 contraction chunks.
"""
import os
import sys
import types

import numpy as np

if "/opt/trn_rl_repo" not in sys.path:
    sys.path.insert(0, "/opt/trn_rl_repo")

import concourse.bacc as bacc
import concourse.tile as tile
from concourse import mybir
from concourse.bass_utils import run_bass_kernel_spmd
from concourse.masks import make_identity

F32 = mybir.dt.float32
F32R = mybir.dt.float32r
BF16 = mybir.dt.bfloat16
EXP = mybir.ActivationFunctionType.Exp
COPY = mybir.ActivationFunctionType.Copy

B, N, D = 2, 2048, 2048
H, KV, DH = 32, 8, 64
G = H // KV                      # 4 q-heads per kv head
HPC, KVPC = 8, 2                 # heads / kv-heads per core
DQ = HPC * DH                    # 512 per-core q projection width
NT = N // 128                    # 16 row tiles
NBW = 512                        # q-block width for attention
NB = N // NBW                    # 4 q-blocks
DC = D // 128                    # 16 contraction chunks

_CACHED = {}


def _build():
    nc = bacc.Bacc("TRN2", target_bir_lowering=False, debug=False, num_devices=8)

    xT = nc.dram_tensor("xT", [D, N], F32R, kind="ExternalInput")
    Wq = nc.dram_tensor("Wq", [D, DQ], F32R, kind="ExternalInput")
    Wk = nc.dram_tensor("Wk", [D, KVPC * DH], F32R, kind="ExternalInput")
    Wv = nc.dram_tensor("Wv", [D, KVPC * DH], F32R, kind="ExternalInput")
    Wo = nc.dram_tensor("Wo", [DQ, D], F32R, kind="ExternalInput")
    OUT = nc.dram_tensor("out", [N, D], F32, kind="ExternalOutput")

    with tile.TileContext(nc) as tc:
        with (
            tc.tile_pool(name="persist", bufs=1) as pp,
            tc.tile_pool(name="wbig", bufs=16) as wbig,
            tc.tile_pool(name="wkv", bufs=16) as wkvp,
            tc.tile_pool(name="xs", bufs=4) as xsp,
            tc.tile_pool(name="vt", bufs=2) as vtp,
            tc.tile_pool(name="pt", bufs=4) as ptp,
            tc.tile_pool(name="outs", bufs=2) as outp,
            tc.tile_pool(name="small", bufs=3) as smp,
            tc.tile_pool(name="ps", bufs=8, space="PSUM") as psp,
        ):
            # ---- persistent sbuf state ----
            ident = pp.tile([128, 128], F32, tag="ident")
            make_identity(nc, ident[:])
            # lower-triangle-in-column-sense mask: mask[r, j] = 1 if j >= r
            tri = pp.tile([128, 128], F32, tag="tri")
            nc.gpsimd.memset(tri[:], 1.0)
            # iota = j - r; where j - r >= 0 keep in_ (1), else fill 0
            nc.gpsimd.affine_select(
                out=tri[:], in_=tri[:],
                compare_op=mybir.AluOpType.is_ge,
                fill=0.0, base=0,
                pattern=[[1, 128]],
                channel_multiplier=-1,
            )

            ones_f = pp.tile([128, DH], F32, tag="onesf")
            nc.vector.memset(ones_f[:], 1.0)
            ones_t = pp.tile([128, DH], F32R, tag="ones")
            nc.vector.tensor_copy(ones_t[:], ones_f[:])

            qt = [pp.tile([128, N], F32R, tag=f"qt{s}", name=f"qt{s}")
                  for s in range(4)]
            kt = pp.tile([128, N], F32R, tag="kt")
            vaug = [pp.tile([128, 2 * (DH + 1)], F32R, tag=f"va{m}", name=f"va{m}")
                    for m in range(NT)]
            ctxT = [pp.tile([128, N], F32R, tag=f"ct{j}", name=f"ct{j}")
                    for j in range(4)]

            # ---- load weights ----
            wq_sb = []
            for dc in range(DC):
                t = wbig.tile([128, DQ], F32R, tag="w")
                nc.scalar.dma_start(out=t[:], in_=Wq[dc * 128:(dc + 1) * 128, :])
                wq_sb.append(t)
            wk_sb, wv_sb = [], []
            for dc in range(DC):
                t = wkvp.tile([128, KVPC * DH], F32R, tag="wk")
                nc.scalar.dma_start(out=t[:], in_=Wk[dc * 128:(dc + 1) * 128, :])
                wk_sb.append(t)
            for dc in range(DC):
                t = wkvp.tile([128, KVPC * DH], F32R, tag="wv")
                nc.scalar.dma_start(out=t[:], in_=Wv[dc * 128:(dc + 1) * 128, :])
                wv_sb.append(t)

            # ---- phase 1: projections, streaming xT once ----
            for nb in range(NB):
                ncol = slice(nb * NBW, (nb + 1) * NBW)
                q_ps = [psp.tile([128, NBW], F32, tag="ps", name=f"qps{_}")
                        for _ in range(4)]
                k_ps = psp.tile([128, NBW], F32, tag="ps", name="kps")
                v_ps = psp.tile([128, NBW], F32, tag="ps", name="vps")
                for dc in range(DC):
                    xs = xsp.tile([128, NBW], F32R, tag="xs")
                    nc.sync.dma_start(out=xs[:],
                                      in_=xT[dc * 128:(dc + 1) * 128, ncol])
                    st, sp = dc == 0, dc == DC - 1
                    for s in range(4):
                        nc.tensor.matmul(q_ps[s][:],
                                         wq_sb[dc][:, s * 128:(s + 1) * 128],
                                         xs[:], start=st, stop=sp)
                    nc.tensor.matmul(k_ps[:], wk_sb[dc][:], xs[:],
                                     start=st, stop=sp)
                    nc.tensor.matmul(v_ps[:], wv_sb[dc][:], xs[:],
                                     start=st, stop=sp)
                for s in range(4):
                    nc.vector.tensor_copy(qt[s][:, ncol], q_ps[s][:])
                nc.vector.tensor_copy(kt[:, ncol], k_ps[:])
                # V^T -> V via PE transpose, split the 2 kv heads around the
                # ones columns of vaug ([0:64]=A, 64=ones, [65:129]=B, 129=ones)
                vts = vtp.tile([128, NBW], F32, tag="vts")
                nc.vector.tensor_copy(vts[:], v_ps[:])
                for i in range(NBW // 128):
                    mt = nb * (NBW // 128) + i
                    tp = psp.tile([128, 128], F32, tag="ps", name="tps")
                    nc.tensor.transpose(tp[:], vts[:, i * 128:(i + 1) * 128],
                                        ident[:])
                    nc.vector.tensor_copy(vaug[mt][:, 0:DH], tp[:, 0:DH])
                    nc.vector.tensor_copy(vaug[mt][:, DH + 1:2 * DH + 1],
                                          tp[:, DH:2 * DH])
                    nc.vector.tensor_copy(vaug[mt][:, DH:DH + 1],
                                          ones_f[:, 0:1])
                    nc.vector.tensor_copy(vaug[mt][:, 2 * DH + 1:2 * DH + 2],
                                          ones_f[:, 0:1])

            # ---- phase 2: attention ----
            scale = 1.0 / np.sqrt(DH)

            def emit_norm(c_ps, j, par, q0):
                # normalize: ctx^T rows /= row 64 (the ones-col sums).
                # Sums sit on psum partition 64; engines cannot shift
                # partitions, so broadcast to partitions 0:64 with a K=1
                # ones matmul, then reciprocal.
                lrow = smp.tile([128, NBW], F32R, tag="lrow", name="lrow")
                nc.vector.tensor_copy(lrow[DH:DH + 1, :], c_ps[DH:DH + 1, :])
                rb_ps = psp.tile([DH, NBW], F32, tag="ps", name="rbps")
                nc.tensor.matmul(rb_ps[:], ones_t[DH:DH + 1, 0:DH],
                                 lrow[DH:DH + 1, :], start=True, stop=True)
                rb = smp.tile([DH, NBW], F32, tag="rb", name="rb")
                nc.vector.reciprocal(rb[:], rb_ps[:])
                if par == 0:
                    nc.vector.tensor_mul(ctxT[j][0:DH, q0:q0 + NBW],
                                         c_ps[0:DH, :], rb[:])
                else:
                    tmp = smp.tile([DH, NBW], F32R, tag="ctmp", name="ctmp")
                    nc.vector.tensor_mul(tmp[:], c_ps[0:DH, :], rb[:])
                    nc.sync.dma_start(out=ctxT[j][DH:2 * DH, q0:q0 + NBW],
                                      in_=tmp[:])

            # software-pipelined: ctx matmuls trail their exp by one scores
            # matmul so the in-order PE queue never head-blocks on ACT.
            blocks = [(hh, nb) for hh in range(HPC) for nb in range(NB)]
            finish_prev = None
            for hh, nb in blocks:
                kv, g = hh // G, hh % G
                j, par = hh // 2, hh % 2
                krows = slice(kv * 64, kv * 64 + 64)
                q0 = nb * NBW
                c_ps = psp.tile([DH + 1, NBW], F32, tag="ps", name="cps")
                n_mb = 4 * nb + 4
                pend_ctx = None
                for mb in range(n_mb):
                    m0 = mb * 128
                    off = max(0, m0 - q0)       # local col offset
                    w = NBW - off
                    s_ps = psp.tile([128, NBW], F32, tag="ps", name="sps")
                    nc.tensor.matmul(
                        s_ps[:, 0:w],
                        kt[krows, m0:m0 + 128],
                        qt[g][krows, q0 + off:q0 + NBW],
                        start=True, stop=True)
                    p_sb = ptp.tile([128, NBW], F32R, tag="pt", name="pt")
                    nc.scalar.activation(p_sb[:, 0:w], s_ps[:, 0:w], EXP,
                                         scale=float(scale))
                    if mb >= 4 * nb:  # diagonal block: triangle mask
                        nc.vector.tensor_mul(p_sb[:, 0:128],
                                             p_sb[:, 0:128], tri[:])
                    if mb == 0 and finish_prev is not None:
                        finish_prev()
                        finish_prev = None
                    if pend_ctx is not None:
                        pend_ctx()

                    def _ctx(c_ps=c_ps, p_sb=p_sb, off=off, w=w, mb=mb,
                             kv=kv, n_mb=n_mb):
                        nc.tensor.matmul(
                            c_ps[:, off:NBW],
                            vaug[mb][:, kv * (DH + 1):(kv + 1) * (DH + 1)],
                            p_sb[:, 0:w],
                            start=(mb == 0), stop=(mb == n_mb - 1))
                    pend_ctx = _ctx

                def _fin(pend_ctx=pend_ctx, c_ps=c_ps, j=j, par=par, q0=q0):
                    pend_ctx()
                    emit_norm(c_ps, j, par, q0)
                finish_prev = _fin
            finish_prev()

            # ---- phase 3: out projection ----
            wo_sb = {}
            for j in range(4):
                for ob in range(4):
                    t = wbig.tile([128, NBW], F32R, tag="w")
                    nc.sync.dma_start(
                        out=t[:],
                        in_=Wo[j * 128:(j + 1) * 128, ob * NBW:(ob + 1) * NBW])
                    wo_sb[(j, ob)] = t
            for nt in range(NT):
                o_sb = outp.tile([128, D], F32, tag="osb")
                for ob in range(4):
                    o_ps = psp.tile([128, NBW], F32, tag="ps", name="ops")
                    for j in range(4):
                        nc.tensor.matmul(o_ps[:],
                                         ctxT[j][:, nt * 128:(nt + 1) * 128],
                                         wo_sb[(j, ob)][:],
                                         start=(j == 0), stop=(j == 3))
                    nc.vector.tensor_copy(o_sb[:, ob * NBW:(ob + 1) * NBW],
                                          o_ps[:])
                nc.sync.dma_start(out=OUT[nt * 128:(nt + 1) * 128, :],
                                  in_=o_sb[:])

    nc.compile()
    return nc


def kernel(x, Wq, Wk, Wv, Wo, bo):
    x = np.asarray(x, dtype=np.float32)
    Wq = np.asarray(Wq, dtype=np.float32)
    Wk = np.asarray(Wk, dtype=np.float32)
    Wv = np.asarray(Wv, dtype=np.float32)
    Wo = np.asarray(Wo, dtype=np.float32)
    bo = np.asarray(bo, dtype=np.float32)

    if "nc" not in _CACHED:
        _CACHED["nc"] = _build()
    nc = _CACHED["nc"]

    in_maps = []
    for c in range(8):
        b, t = c // 4, c % 4
        xT = np.ascontiguousarray(x[b].T)
        # q slab s holds [kv-head 2t head g=s | kv-head 2t+1 head g=s]
        qcols = []
        for s in range(4):
            for kvl in range(KVPC):
                h = (2 * t + kvl) * G + s
                qcols.append(Wq[:, h * DH:(h + 1) * DH])
        wq_c = np.ascontiguousarray(np.concatenate(qcols, axis=1))
        wk_c = np.ascontiguousarray(Wk[:, t * 128:(t + 1) * 128])
        wv_c = np.ascontiguousarray(Wv[:, t * 128:(t + 1) * 128])
        wo_c = np.ascontiguousarray(Wo[t * DQ:(t + 1) * DQ, :])
        in_maps.append({"xT": xT, "Wq": wq_c, "Wk": wk_c, "Wv": wv_c,
                        "Wo": wo_c})

    trace = bool(int(os.environ.get("GQA_TRACE", "0")))
    kwargs = {}
    if trace:
        import tempfile
        td = os.environ.get("GQA_TRACE_DIR") or tempfile.mkdtemp(prefix="gqa_")
        kwargs = dict(trace=True, tmpdir=td)
    res = run_bass_kernel_spmd(nc, in_maps, list(range(8)), **kwargs)
    _CACHED["last_result"] = res

    out = np.empty((B, N, D), dtype=np.float32)
    for b in range(B):
        acc = res.results[4 * b]["out"].astype(np.float32)
        for t in range(1, 4):
            acc = acc + res.results[4 * b + t]["out"]
        out[b] = acc + bo[None, :]
    return out



# revision 4
# speedup vs baseline: 1.2205x; 1.2205x over previous
"""GQA forward (B=2,N=2048,D=2048,H=32,KV=8,DH=64, causal) on 8 trn2 cores.

Sharding: 2-way data parallel over batch x 4-way tensor parallel over heads
(each core: 8 q-heads = 2 kv-heads, keeping group structure). Row-parallel
out-proj; the all-reduce over the 4 TP shards (+ bias) happens on host at
gather time.

v2 design (vs baseline three serial phases):
  - all matmul operands bf16 (fp32 PSUM accumulation) -> FWL weight loads,
    half DMA/SBUF traffic, no fp32r narrow-moving penalty.
  - one fused pipeline: projection of q-block b+1 and out-projection of
    q-block b-1 are interleaved as PE filler between the attention matmuls
    of q-block b, so the tensor engine never idles long enough for the HAM
    clock gate to re-throttle to 1.2 GHz (the baseline spent 389us at half
    clock during attention).
  - scores for the 2 kv-heads of a head-pair run concurrently in PE row
    groups (K=64 contractions at base partitions 0 / 64).
  - causal mask applied by accumulating an identity-matmul of a -30000
    constant onto the diagonal score blocks (no DVE in the exp->ctx path).
  - exp batched: one ACT instruction per [128, 1024] PSUM span (both heads
    of a pair for one key block).
  - V projected directly in [tokens, dh] orientation with xs chunks as the
    stationary operand (no PE transposes).
"""
import os
import sys

import numpy as np

if "/opt/trn_rl_repo" not in sys.path:
    sys.path.insert(0, "/opt/trn_rl_repo")

import ml_dtypes

import concourse.bacc as bacc
import concourse.tile as tile
from concourse import mybir
from concourse.bass_utils import run_bass_kernel_spmd
from concourse.masks import make_identity

F32 = mybir.dt.float32
F32R = mybir.dt.float32r
BF16 = mybir.dt.bfloat16
EXP = mybir.ActivationFunctionType.Exp

B, N, D = 2, 2048, 2048
H, KV, DH = 32, 8, 64
G = H // KV                      # 4 q-heads per kv head
HPC, KVPC = 8, 2                 # heads / kv-heads per core
DQ = HPC * DH                    # 512 per-core q projection width
NBW = 512                        # q-block width
NB = N // NBW                    # 4 q-blocks
DC = D // 128                    # 16 contraction chunks
NEG = -30000.0                   # causal mask additive constant

_CACHED = {}


def _build():
    nc = bacc.Bacc("TRN2", target_bir_lowering=False, debug=False,
                   num_devices=8)

    xT = nc.dram_tensor("xT", [D, N], BF16, kind="ExternalInput")
    Wq = nc.dram_tensor("Wq", [D, DQ], BF16, kind="ExternalInput")
    Wk = nc.dram_tensor("Wk", [D, KVPC * DH], BF16, kind="ExternalInput")
    Wv = nc.dram_tensor("Wv", [D, KVPC * DH], BF16, kind="ExternalInput")
    Wo = nc.dram_tensor("Wo", [DQ, D], BF16, kind="ExternalInput")
    OUT = nc.dram_tensor("out", [N, D], F32, kind="ExternalOutput")

    scale = 1.0 / np.sqrt(DH)

    with tile.TileContext(nc) as tc:
        with (
            tc.tile_pool(name="persist", bufs=1) as pp,
            tc.tile_pool(name="wq", bufs=16) as wqp,
            tc.tile_pool(name="wkv", bufs=32) as wkvp,
            tc.tile_pool(name="wo", bufs=16) as wop,
            tc.tile_pool(name="xs", bufs=16) as xsp,
            tc.tile_pool(name="psb", bufs=4) as psbp,
            tc.tile_pool(name="outs", bufs=2) as outp,
            tc.tile_pool(name="small", bufs=4) as smp,
            tc.tile_pool(name="sc_ps", bufs=2, space="PSUM") as scp,
            tc.tile_pool(name="c_ps", bufs=2, space="PSUM") as cpp,
            tc.tile_pool(name="pj_ps", bufs=1, space="PSUM") as pjp,
            tc.tile_pool(name="ms_ps", bufs=1, space="PSUM") as msp,
        ):
            # ---- persistent sbuf state ----
            identf = pp.tile([128, 128], F32, tag="identf")
            make_identity(nc, identf[:])
            ident = pp.tile([128, 128], BF16, tag="ident")
            nc.vector.tensor_copy(ident[:], identf[:])

            # additive causal mask for a 128x128 diagonal block:
            # mask[k, j] = 0 if j >= k else NEG (local query j, local key k)
            mknf = pp.tile([128, 128], F32, tag="mknf")
            nc.gpsimd.memset(mknf[:], 0.0)
            nc.gpsimd.affine_select(
                out=mknf[:], in_=mknf[:],
                compare_op=mybir.AluOpType.is_ge,
                fill=NEG, base=0,
                pattern=[[1, 128]],
                channel_multiplier=-1,
            )
            maskneg = pp.tile([128, 128], BF16, tag="maskneg")
            nc.vector.tensor_copy(maskneg[:], mknf[:])

            onesf = pp.tile([128, 64], F32, tag="onesf")
            nc.vector.memset(onesf[:], 1.0)
            ones_b = pp.tile([128, 64], BF16, tag="onesb")
            nc.vector.tensor_copy(ones_b[:], onesf[:])
            ones_r = pp.tile([128, 64], F32R, tag="onesr")
            nc.vector.tensor_copy(ones_r[:], onesf[:])

            qt = [pp.tile([128, N], BF16, tag=f"qt{g}", name=f"qt{g}")
                  for g in range(4)]
            kt = pp.tile([128, N], BF16, tag="kt")
            # va[m]: [ A_dh(0:64) | onesA(64) | B_dh(65:129) | onesB(129) ]
            va = [pp.tile([128, 2 * (DH + 1)], BF16, tag=f"va{m}",
                          name=f"va{m}") for m in range(N // 128)]
            for m in range(N // 128):
                nc.vector.tensor_copy(va[m][:, DH:DH + 1], ones_b[:, 0:1])
                nc.vector.tensor_copy(va[m][:, 2 * DH + 1:2 * DH + 2],
                                      ones_b[:, 0:1])
            ctxT = [pp.tile([128, N], BF16, tag=f"ct{g}", name=f"ct{g}")
                    for g in range(4)]

            # ---- weights ----
            wq_sb, wk_sb, wv_sb = [], [], []
            for dc in range(DC):
                t = wqp.tile([128, DQ], BF16, tag="w")
                nc.scalar.dma_start(out=t[:], in_=Wq[dc * 128:(dc + 1) * 128, :])
                wq_sb.append(t)
            for dc in range(DC):
                t = wkvp.tile([128, KVPC * DH], BF16, tag="wk")
                nc.scalar.dma_start(out=t[:], in_=Wk[dc * 128:(dc + 1) * 128, :])
                wk_sb.append(t)
            for dc in range(DC):
                t = wkvp.tile([128, KVPC * DH], BF16, tag="wv")
                nc.scalar.dma_start(out=t[:], in_=Wv[dc * 128:(dc + 1) * 128, :])
                wv_sb.append(t)
            wo_sb = {}
            for j in range(4):
                for ob in range(4):
                    t = wop.tile([128, NBW], BF16, tag="wo")
                    nc.sync.dma_start(
                        out=t[:],
                        in_=Wo[j * 128:(j + 1) * 128, ob * NBW:(ob + 1) * NBW])
                    wo_sb[(j, ob)] = t

            # ---- filler machinery ----------------------------------------
            # Thunks emitting PE-heavy work between attention ops so the
            # tensor engine never starves while ACT runs exp. hi = next
            # block's projections (must drain before that block's attention);
            # lo = previous block's out-projection (can carry over stages).
            filler_hi = []
            filler_lo = []

            def emit_filler(n):
                for _ in range(n):
                    if filler_hi:
                        filler_hi.pop(0)()
                    elif filler_lo:
                        filler_lo.pop(0)()
                    else:
                        break

            def drain_hi():
                while filler_hi:
                    filler_hi.pop(0)()

            xs_tiles = {}          # b -> list of 16 sbuf tiles

            def emit_xs_dma(b):
                ts = []
                for dc in range(DC):
                    t = xsp.tile([128, NBW], BF16, tag="xs", name="xs")
                    nc.sync.dma_start(
                        out=t[:],
                        in_=xT[dc * 128:(dc + 1) * 128,
                               b * NBW:(b + 1) * NBW])
                    ts.append(t)
                xs_tiles[b] = ts

            def proj_thunks(b):
                """Projection of q-block b: 6 bank-passes over resident xs,
                each pass split into 4-dc chunks (~850ns PE) for fine
                interleaving."""
                xs = xs_tiles[b]
                thunks = []

                def qk_chunk(g, c0, cell):
                    # g in 0..3 -> q slab g ; g == 4 -> k
                    def mk():
                        if c0 == 0:
                            cell.append(
                                pjp.tile([128, NBW], F32, tag="pj", name="pj"))
                        ps = cell[0]
                        for dc in range(c0, c0 + 4):
                            stat = (wq_sb[dc][:, g * 128:(g + 1) * 128]
                                    if g < 4 else wk_sb[dc][:])
                            nc.tensor.matmul(ps[:], stat, xs[dc][:],
                                             start=(dc == 0),
                                             stop=(dc == DC - 1))
                        if c0 + 4 == DC:
                            dst = qt[g] if g < 4 else kt
                            nc.vector.tensor_copy(
                                dst[:, b * NBW:(b + 1) * NBW], ps[:])
                    return mk

                def v_chunk(c0, cell):
                    def mk():
                        if c0 == 0:
                            cell.append(
                                pjp.tile([128, NBW], F32, tag="pj", name="pj"))
                        ps = cell[0]
                        for dc in range(c0, c0 + 4):
                            for i in range(4):
                                nc.tensor.matmul(
                                    ps[:, i * 128:(i + 1) * 128],
                                    xs[dc][:, i * 128:(i + 1) * 128],
                                    wv_sb[dc][:],
                                    start=(dc == 0 and i == 0),
                                    stop=(dc == DC - 1 and i == 3),
                                    skip_group_check=True)
                        if c0 + 4 == DC:
                            for i in range(4):
                                m = b * 4 + i
                                nc.vector.tensor_copy(
                                    va[m][:, 0:DH],
                                    ps[:, i * 128:i * 128 + 64])
                                nc.vector.tensor_copy(
                                    va[m][:, DH + 1:2 * DH + 1],
                                    ps[:, i * 128 + 64:i * 128 + 128])
                    return mk

                for g in range(5):
                    cell = []
                    for c0 in range(0, DC, 4):
                        thunks.append(qk_chunk(g, c0, cell))
                cell = []
                for c0 in range(0, DC, 4):
                    thunks.append(v_chunk(c0, cell))
                return thunks

            def outproj_thunks(b):
                """Out-projection of q-block b (4 row tiles)."""
                thunks = []

                def ob_unit(nt, ob, osb):
                    def mk():
                        ops = msp.tile([128, NBW], F32, tag="ms", name="ops")
                        for j in range(4):
                            nc.tensor.matmul(
                                ops[:],
                                ctxT[j][:, nt * 128:(nt + 1) * 128],
                                wo_sb[(j, ob)][:],
                                start=(j == 0), stop=(j == 3))
                        nc.vector.tensor_copy(
                            osb[0][:, ob * NBW:(ob + 1) * NBW], ops[:])
                    return mk

                def out_dma(nt, osb):
                    def mk():
                        nc.sync.dma_start(
                            out=OUT[nt * 128:(nt + 1) * 128, :], in_=osb[0][:])
                    return mk

                for i in range(4):
                    nt = b * 4 + i
                    osb = []

                    def alloc(osb=osb):
                        osb.append(outp.tile([128, D], F32, tag="osb",
                                             name="osb"))
                    thunks.append(alloc)
                    for ob in range(4):
                        thunks.append(ob_unit(nt, ob, osb))
                    thunks.append(out_dma(nt, osb))
                return thunks

            # ---- norm: ctx / softmax-denominator, into ctxT ---------------
            def emit_norm(c_ps, g, par, q0):
                # denominator sits on psum partition 64 (the ones column of
                # va). Broadcast to partitions 0:64 via a K=1 ones matmul,
                # reciprocal, multiply on the PSUM->SBUF move. Head B's 64
                # ctx rows reach ctxT partitions 64:128 via a sbuf-to-sbuf
                # DMA (engines cannot shift partitions).
                lrow = smp.tile([65, NBW], F32R, tag="lrow", name="lrow")
                nc.vector.tensor_copy(lrow[DH:DH + 1, :], c_ps[DH:DH + 1, :])
                rb_ps = msp.tile([DH, NBW], F32, tag="ms", name="rbps")
                nc.tensor.matmul(rb_ps[:], ones_r[DH:DH + 1, 0:DH],
                                 lrow[DH:DH + 1, :], start=True, stop=True)
                rb = smp.tile([DH, NBW], F32, tag="rb", name="rb")
                nc.vector.reciprocal(rb[:], rb_ps[:])
                if par == 0:
                    nc.vector.tensor_mul(ctxT[g][0:DH, q0:q0 + NBW],
                                         c_ps[0:DH, :], rb[:])
                else:
                    tmp = smp.tile([DH, NBW], BF16, tag="ctmp", name="ctmp")
                    nc.vector.tensor_mul(tmp[:], c_ps[0:DH, :], rb[:])
                    nc.sync.dma_start(out=ctxT[g][DH:2 * DH, q0:q0 + NBW],
                                      in_=tmp[:])

            # ---- attention for one q-block, with filler interleave --------
            def attention(b):
                q0 = b * NBW
                n_kb = 4 * b + 4
                for g in range(4):
                    c_a = cpp.tile([DH + 1, NBW], F32, tag="c", name="ca")
                    c_b = cpp.tile([DH + 1, NBW], F32, tag="c", name="cb")
                    pend = None
                    for kb in range(n_kb):
                        m0 = kb * 128
                        diag = kb >= 4 * b
                        off = max(0, m0 - q0)
                        T = scp.tile([128, 2 * NBW], F32, tag="sc", name="T")
                        # scores: kv0 rows 0:64 / kv1 rows 64:128 run as
                        # concurrent PE row-groups
                        nc.tensor.matmul(T[:, 0:NBW],
                                         kt[0:64, m0:m0 + 128],
                                         qt[g][0:64, q0:q0 + NBW],
                                         start=True, stop=not diag,
                                         skip_group_check=True)
                        nc.tensor.matmul(T[:, NBW:2 * NBW],
                                         kt[64:128, m0:m0 + 128],
                                         qt[g][64:128, q0:q0 + NBW],
                                         start=True, stop=not diag,
                                         skip_group_check=True)
                        if diag:
                            nc.tensor.matmul(T[:, off:off + 128],
                                             ident[:], maskneg[:],
                                             start=False, stop=True,
                                             skip_group_check=True)
                            nc.tensor.matmul(T[:, NBW + off:NBW + off + 128],
                                             ident[:], maskneg[:],
                                             start=False, stop=True,
                                             skip_group_check=True)
                        p = psbp.tile([128, 2 * NBW], BF16, tag="p", name="p")
                        nc.scalar.activation(p[:], T[:], EXP,
                                             scale=float(scale))
                        if pend is not None:
                            pend()
                        emit_filler(1)

                        def _ctx(kb=kb, off=off, p=p, c_a=c_a, c_b=c_b):
                            nc.tensor.matmul(
                                c_a[:, off:NBW],
                                va[kb][:, 0:DH + 1],
                                p[:, off:NBW],
                                start=(kb == 0), stop=(kb == n_kb - 1),
                                skip_group_check=True)
                            nc.tensor.matmul(
                                c_b[:, off:NBW],
                                va[kb][:, DH + 1:2 * DH + 2],
                                p[:, NBW + off:2 * NBW],
                                start=(kb == 0), stop=(kb == n_kb - 1),
                                skip_group_check=True)
                        pend = _ctx
                    pend()
                    emit_norm(c_a, g, 0, q0)
                    emit_norm(c_b, g, 1, q0)
                    emit_filler(2)

            # ---- main schedule -------------------------------------------
            emit_xs_dma(0)
            for th in proj_thunks(0):
                th()
            emit_xs_dma(1)
            for b in range(NB):
                if b + 1 < NB:
                    filler_hi.extend(proj_thunks(b + 1))
                if b + 2 < NB:
                    filler_hi.append(lambda b=b: emit_xs_dma(b + 2))
                if b >= 1:
                    filler_lo.extend(outproj_thunks(b - 1))
                attention(b)
                drain_hi()
            filler_lo.extend(outproj_thunks(NB - 1))
            while filler_lo:
                filler_lo.pop(0)()

    nc.compile()
    return nc


def kernel(x, Wq, Wk, Wv, Wo, bo):
    x = np.asarray(x, dtype=np.float32)
    Wq = np.asarray(Wq, dtype=np.float32)
    Wk = np.asarray(Wk, dtype=np.float32)
    Wv = np.asarray(Wv, dtype=np.float32)
    Wo = np.asarray(Wo, dtype=np.float32)
    bo = np.asarray(bo, dtype=np.float32)

    if "nc" not in _CACHED:
        _CACHED["nc"] = _build()
    nc = _CACHED["nc"]

    bf = ml_dtypes.bfloat16
    in_maps = []
    for c in range(8):
        b, t = c // 4, c % 4
        xTc = np.ascontiguousarray(x[b].T).astype(bf)
        # q slab g holds [kv-head 2t head g | kv-head 2t+1 head g]
        qcols = []
        for g in range(4):
            for kvl in range(KVPC):
                h = (2 * t + kvl) * G + g
                qcols.append(Wq[:, h * DH:(h + 1) * DH])
        wq_c = np.ascontiguousarray(np.concatenate(qcols, axis=1)).astype(bf)
        wk_c = np.ascontiguousarray(Wk[:, t * 128:(t + 1) * 128]).astype(bf)
        wv_c = np.ascontiguousarray(Wv[:, t * 128:(t + 1) * 128]).astype(bf)
        # Wo rows must follow the ctxT head-pair layout: slab g holds
        # [head (kv 2t, g) | head (kv 2t+1, g)]
        wrows = []
        for g in range(4):
            for kvl in range(KVPC):
                h = (2 * t + kvl) * G + g
                wrows.append(Wo[h * DH:(h + 1) * DH, :])
        wo_c = np.ascontiguousarray(np.concatenate(wrows, axis=0)).astype(bf)
        in_maps.append({"xT": xTc, "Wq": wq_c, "Wk": wk_c, "Wv": wv_c,
                        "Wo": wo_c})

    trace = bool(int(os.environ.get("GQA_TRACE", "0")))
    kwargs = {}
    if trace:
        import tempfile
        td = os.environ.get("GQA_TRACE_DIR") or tempfile.mkdtemp(prefix="gqa_")
        kwargs = dict(trace=True, tmpdir=td)
    res = run_bass_kernel_spmd(nc, in_maps, list(range(8)), **kwargs)
    _CACHED["last_result"] = res

    out = np.empty((B, N, D), dtype=np.float32)
    for b in range(B):
        acc = res.results[4 * b]["out"].astype(np.float32)
        for t in range(1, 4):
            acc = acc + res.results[4 * b + t]["out"]
        out[b] = acc + bo[None, :]
    return out


# revision 7
# speedup vs baseline: 1.2256x; 1.0042x over previous
"""GQA forward (B=2,N=2048,D=2048,H=32,KV=8,DH=64, causal) on 8 trn2 cores.

Sharding: 2-way data parallel over batch x 4-way tensor parallel over heads
(each core: 8 q-heads = 2 kv-heads, keeping group structure). Row-parallel
out-proj; the all-reduce over the 4 TP shards (+ bias) happens on host at
gather time.

v2 design (vs baseline three serial phases):
  - all matmul operands bf16 (fp32 PSUM accumulation) -> FWL weight loads,
    half DMA/SBUF traffic, no fp32r narrow-moving penalty.
  - one fused pipeline: projection of q-block b+1 and out-projection of
    q-block b-1 are interleaved as PE filler between the attention matmuls
    of q-block b, so the tensor engine never idles long enough for the HAM
    clock gate to re-throttle to 1.2 GHz (the baseline spent 389us at half
    clock during attention).
  - scores for the 2 kv-heads of a head-pair run concurrently in PE row
    groups (K=64 contractions at base partitions 0 / 64).
  - causal mask applied by accumulating an identity-matmul of a -30000
    constant onto the diagonal score blocks (no DVE in the exp->ctx path).
  - exp batched: one ACT instruction per [128, 1024] PSUM span (both heads
    of a pair for one key block).
  - V projected directly in [tokens, dh] orientation with xs chunks as the
    stationary operand (no PE transposes).
"""
import os
import sys

import numpy as np

if "/opt/trn_rl_repo" not in sys.path:
    sys.path.insert(0, "/opt/trn_rl_repo")

import ml_dtypes

import concourse.bacc as bacc
import concourse.tile as tile
from concourse import mybir
from concourse.bass_utils import run_bass_kernel_spmd
from concourse.masks import make_identity

F32 = mybir.dt.float32
F32R = mybir.dt.float32r
BF16 = mybir.dt.bfloat16
EXP = mybir.ActivationFunctionType.Exp
LN = mybir.ActivationFunctionType.Ln

B, N, D = 2, 2048, 2048
H, KV, DH = 32, 8, 64
G = H // KV                      # 4 q-heads per kv head
HPC, KVPC = 8, 2                 # heads / kv-heads per core
DQ = HPC * DH                    # 512 per-core q projection width
NBW = 512                        # q-block width
NB = N // NBW                    # 4 q-blocks
DC = D // 128                    # 16 contraction chunks
NEG = -30000.0                   # causal mask additive constant

_CACHED = {}


def _build():
    nc = bacc.Bacc("TRN2", target_bir_lowering=False, debug=False,
                   num_devices=8)

    xT = nc.dram_tensor("xT", [D, N], BF16, kind="ExternalInput")
    Wq = nc.dram_tensor("Wq", [D, DQ], BF16, kind="ExternalInput")
    Wk = nc.dram_tensor("Wk", [D, KVPC * DH], BF16, kind="ExternalInput")
    Wv = nc.dram_tensor("Wv", [D, KVPC * DH], BF16, kind="ExternalInput")
    Wo = nc.dram_tensor("Wo", [DQ, D], BF16, kind="ExternalInput")
    OUT = nc.dram_tensor("out", [N, D], F32, kind="ExternalOutput")

    scale = 1.0 / np.sqrt(DH)

    with tile.TileContext(nc) as tc:
        with (
            tc.tile_pool(name="persist", bufs=1) as pp,
            tc.tile_pool(name="wq", bufs=16) as wqp,
            tc.tile_pool(name="wkv", bufs=32) as wkvp,
            tc.tile_pool(name="wo", bufs=16) as wop,
            tc.tile_pool(name="xs", bufs=16) as xsp,
            tc.tile_pool(name="psb", bufs=4) as psbp,
            tc.tile_pool(name="outs", bufs=2) as outp,
            tc.tile_pool(name="small", bufs=4) as smp,
            tc.tile_pool(name="sc_ps", bufs=2, space="PSUM") as scp,
            tc.tile_pool(name="c_ps", bufs=2, space="PSUM") as cpp,
            tc.tile_pool(name="pj_ps", bufs=1, space="PSUM") as pjp,
            tc.tile_pool(name="ms_ps", bufs=1, space="PSUM") as msp,
        ):
            # ---- persistent sbuf state ----
            identf = pp.tile([128, 128], F32, tag="identf")
            make_identity(nc, identf[:])
            ident = pp.tile([128, 128], BF16, tag="ident")
            nc.vector.tensor_copy(ident[:], identf[:])

            # additive causal mask for a 128x128 diagonal block:
            # mask[k, j] = 0 if j >= k else NEG (local query j, local key k)
            mknf = pp.tile([128, 128], F32, tag="mknf")
            nc.gpsimd.memset(mknf[:], 0.0)
            nc.gpsimd.affine_select(
                out=mknf[:], in_=mknf[:],
                compare_op=mybir.AluOpType.is_ge,
                fill=NEG, base=0,
                pattern=[[1, 128]],
                channel_multiplier=-1,
            )
            maskneg = pp.tile([128, 128], BF16, tag="maskneg")
            nc.vector.tensor_copy(maskneg[:], mknf[:])

            onesf = pp.tile([128, 64], F32, tag="onesf")
            nc.vector.memset(onesf[:], 1.0)
            ones_b = pp.tile([128, 64], BF16, tag="onesb")
            nc.vector.tensor_copy(ones_b[:], onesf[:])
            ones_r = pp.tile([128, 64], F32R, tag="onesr")
            nc.vector.tensor_copy(ones_r[:], onesf[:])

            qt = [pp.tile([128, N], BF16, tag=f"qt{g}", name=f"qt{g}")
                  for g in range(4)]
            kt = pp.tile([128, N], BF16, tag="kt")
            # va[m]: [ A_dh(0:64) | onesA(64) | B_dh(65:129) | onesB(129) ]
            va = [pp.tile([128, 2 * (DH + 1)], BF16, tag=f"va{m}",
                          name=f"va{m}") for m in range(N // 128)]
            for m in range(N // 128):
                nc.vector.tensor_copy(va[m][:, DH:DH + 1], ones_b[:, 0:1])
                nc.vector.tensor_copy(va[m][:, 2 * DH + 1:2 * DH + 2],
                                      ones_b[:, 0:1])
            ctxT = [pp.tile([128, N], BF16, tag=f"ct{g}", name=f"ct{g}")
                    for g in range(4)]

            # ---- weights ----
            wq_sb, wk_sb, wv_sb = [], [], []
            for dc in range(DC):
                t = wqp.tile([128, DQ], BF16, tag="w")
                nc.scalar.dma_start(out=t[:], in_=Wq[dc * 128:(dc + 1) * 128, :])
                wq_sb.append(t)
            for dc in range(DC):
                t = wkvp.tile([128, KVPC * DH], BF16, tag="wk")
                nc.scalar.dma_start(out=t[:], in_=Wk[dc * 128:(dc + 1) * 128, :])
                wk_sb.append(t)
            for dc in range(DC):
                t = wkvp.tile([128, KVPC * DH], BF16, tag="wv")
                nc.scalar.dma_start(out=t[:], in_=Wv[dc * 128:(dc + 1) * 128, :])
                wv_sb.append(t)
            wo_sb = {}
            for j in range(4):
                for ob in range(4):
                    t = wop.tile([128, NBW], BF16, tag="wo")
                    nc.sync.dma_start(
                        out=t[:],
                        in_=Wo[j * 128:(j + 1) * 128, ob * NBW:(ob + 1) * NBW])
                    wo_sb[(j, ob)] = t

            # ---- filler machinery ----------------------------------------
            # Thunks emitting PE-heavy work between attention ops so the
            # tensor engine never starves while ACT runs exp. hi = next
            # block's projections (must drain before that block's attention);
            # lo = previous block's out-projection (can carry over stages).
            filler_hi = []
            filler_lo = []

            def emit_filler(n):
                for _ in range(n):
                    if filler_hi:
                        filler_hi.pop(0)()
                    elif filler_lo:
                        filler_lo.pop(0)()
                    else:
                        break

            def drain_hi():
                while filler_hi:
                    filler_hi.pop(0)()

            xs_tiles = {}          # b -> list of 16 sbuf tiles

            def emit_xs_dma(b):
                ts = []
                for dc in range(DC):
                    t = xsp.tile([128, NBW], BF16, tag="xs", name="xs")
                    nc.sync.dma_start(
                        out=t[:],
                        in_=xT[dc * 128:(dc + 1) * 128,
                               b * NBW:(b + 1) * NBW])
                    ts.append(t)
                xs_tiles[b] = ts

            def proj_thunks(b):
                """Projection of q-block b: 6 bank-passes over resident xs,
                each pass split into 4-dc chunks (~850ns PE) for fine
                interleaving."""
                xs = xs_tiles[b]
                thunks = []

                def qk_chunk(g, c0, cell):
                    # g in 0..3 -> q slab g ; g == 4 -> k
                    def mk():
                        if c0 == 0:
                            cell.append(
                                pjp.tile([128, NBW], F32, tag="pj", name="pj"))
                        ps = cell[0]
                        for dc in range(c0, c0 + 4):
                            stat = (wq_sb[dc][:, g * 128:(g + 1) * 128]
                                    if g < 4 else wk_sb[dc][:])
                            nc.tensor.matmul(ps[:], stat, xs[dc][:],
                                             start=(dc == 0),
                                             stop=(dc == DC - 1))
                        if c0 + 4 == DC:
                            dst = qt[g] if g < 4 else kt
                            nc.vector.tensor_copy(
                                dst[:, b * NBW:(b + 1) * NBW], ps[:])
                    return mk

                def v_chunk(c0, cell):
                    def mk():
                        if c0 == 0:
                            cell.append(
                                pjp.tile([128, NBW], F32, tag="pj", name="pj"))
                        ps = cell[0]
                        for dc in range(c0, c0 + 4):
                            for i in range(4):
                                nc.tensor.matmul(
                                    ps[:, i * 128:(i + 1) * 128],
                                    xs[dc][:, i * 128:(i + 1) * 128],
                                    wv_sb[dc][:],
                                    start=(dc == 0 and i == 0),
                                    stop=(dc == DC - 1 and i == 3),
                                    skip_group_check=True)
                        if c0 + 4 == DC:
                            for i in range(4):
                                m = b * 4 + i
                                nc.vector.tensor_copy(
                                    va[m][:, 0:DH],
                                    ps[:, i * 128:i * 128 + 64])
                                nc.vector.tensor_copy(
                                    va[m][:, DH + 1:2 * DH + 1],
                                    ps[:, i * 128 + 64:i * 128 + 128])
                    return mk

                for g in range(5):
                    cell = []
                    for c0 in range(0, DC, 4):
                        thunks.append(qk_chunk(g, c0, cell))
                cell = []
                for c0 in range(0, DC, 4):
                    thunks.append(v_chunk(c0, cell))
                return thunks

            def outproj_thunks(b):
                """Out-projection of q-block b (4 row tiles)."""
                thunks = []

                def ob_unit(nt, ob, osb):
                    def mk():
                        ops = msp.tile([128, NBW], F32, tag="ms", name="ops")
                        for j in range(4):
                            nc.tensor.matmul(
                                ops[:],
                                ctxT[j][:, nt * 128:(nt + 1) * 128],
                                wo_sb[(j, ob)][:],
                                start=(j == 0), stop=(j == 3))
                        nc.vector.tensor_copy(
                            osb[0][:, ob * NBW:(ob + 1) * NBW], ops[:])
                    return mk

                def out_dma(nt, osb):
                    def mk():
                        nc.sync.dma_start(
                            out=OUT[nt * 128:(nt + 1) * 128, :], in_=osb[0][:])
                    return mk

                for i in range(4):
                    nt = b * 4 + i
                    osb = []

                    def alloc(osb=osb):
                        osb.append(outp.tile([128, D], F32, tag="osb",
                                             name="osb"))
                    thunks.append(alloc)
                    for ob in range(4):
                        thunks.append(ob_unit(nt, ob, osb))
                    thunks.append(out_dma(nt, osb))
                return thunks

            # ---- norm: ctx / softmax-denominator, into ctxT ---------------
            def emit_norm(c_ps, g, par, q0):
                # denominator sits on psum partition 64 (the ones column of
                # va). Broadcast to partitions 0:64 via a K=1 ones matmul,
                # reciprocal, multiply on the PSUM->SBUF move. Head B's 64
                # ctx rows reach ctxT partitions 64:128 via a sbuf-to-sbuf
                # DMA (engines cannot shift partitions).
                lrow = smp.tile([65, NBW], F32R, tag="lrow", name="lrow")
                nc.vector.tensor_copy(lrow[DH:DH + 1, :], c_ps[DH:DH + 1, :])
                rb_ps = msp.tile([DH, NBW], F32, tag="ms", name="rbps")
                nc.tensor.matmul(rb_ps[:], ones_r[DH:DH + 1, 0:DH],
                                 lrow[DH:DH + 1, :], start=True, stop=True)
                # 1/denom as exp(-ln(denom)): DVE reciprocal is an iterative
                # divide (~3.3us per [64,512]); ln+exp share one ACT table
                # set with the attention exp, ~0.6us each.
                lg = smp.tile([DH, NBW], F32, tag="lg", name="lg")
                nc.scalar.activation(lg[:], rb_ps[:], LN)
                rb = smp.tile([DH, NBW], F32, tag="rb", name="rb")
                nc.scalar.activation(rb[:], lg[:], EXP, scale=-1.0)
                if par == 0:
                    nc.vector.tensor_mul(ctxT[g][0:DH, q0:q0 + NBW],
                                         c_ps[0:DH, :], rb[:])
                else:
                    tmp = smp.tile([DH, NBW], BF16, tag="ctmp", name="ctmp")
                    nc.vector.tensor_mul(tmp[:], c_ps[0:DH, :], rb[:])
                    nc.sync.dma_start(out=ctxT[g][DH:2 * DH, q0:q0 + NBW],
                                      in_=tmp[:])

            # ---- attention for one q-block, with filler interleave --------
            def attention(b):
                q0 = b * NBW
                n_kb = 4 * b + 4
                for g in range(4):
                    c_a = cpp.tile([DH + 1, NBW], F32, tag="c", name="ca")
                    c_b = cpp.tile([DH + 1, NBW], F32, tag="c", name="cb")
                    pend = None
                    for kb in range(n_kb):
                        m0 = kb * 128
                        diag = kb >= 4 * b
                        off = max(0, m0 - q0)
                        T = scp.tile([128, 2 * NBW], F32, tag="sc", name="T")
                        # scores: kv0 rows 0:64 / kv1 rows 64:128 run as
                        # concurrent PE row-groups. Head A only needs cols
                        # >= off (earlier cols belong to fully-masked
                        # queries); head B keeps full width so the exp span
                        # [off : 2*NBW] reads initialized psum only.
                        nc.tensor.matmul(T[:, off:NBW],
                                         kt[0:64, m0:m0 + 128],
                                         qt[g][0:64, q0 + off:q0 + NBW],
                                         start=True, stop=not diag,
                                         skip_group_check=True)
                        nc.tensor.matmul(T[:, NBW:2 * NBW],
                                         kt[64:128, m0:m0 + 128],
                                         qt[g][64:128, q0:q0 + NBW],
                                         start=True, stop=not diag,
                                         skip_group_check=True)
                        if diag:
                            nc.tensor.matmul(T[:, off:off + 128],
                                             ident[:], maskneg[:],
                                             start=False, stop=True,
                                             skip_group_check=True)
                            nc.tensor.matmul(T[:, NBW + off:NBW + off + 128],
                                             ident[:], maskneg[:],
                                             start=False, stop=True,
                                             skip_group_check=True)
                        p = psbp.tile([128, 2 * NBW], BF16, tag="p", name="p")
                        nc.scalar.activation(p[:, off:2 * NBW],
                                             T[:, off:2 * NBW], EXP,
                                             scale=float(scale))
                        if pend is not None:
                            pend()
                        emit_filler(1)

                        def _ctx(kb=kb, off=off, p=p, c_a=c_a, c_b=c_b):
                            nc.tensor.matmul(
                                c_a[:, off:NBW],
                                va[kb][:, 0:DH + 1],
                                p[:, off:NBW],
                                start=(kb == 0), stop=(kb == n_kb - 1),
                                skip_group_check=True)
                            nc.tensor.matmul(
                                c_b[:, off:NBW],
                                va[kb][:, DH + 1:2 * DH + 2],
                                p[:, NBW + off:2 * NBW],
                                start=(kb == 0), stop=(kb == n_kb - 1),
                                skip_group_check=True)
                        pend = _ctx
                    pend()
                    emit_norm(c_a, g, 0, q0)
                    emit_norm(c_b, g, 1, q0)
                    emit_filler(2)

            # ---- main schedule -------------------------------------------
            emit_xs_dma(0)
            for th in proj_thunks(0):
                th()
            emit_xs_dma(1)
            for b in range(NB):
                if b + 1 < NB:
                    filler_hi.extend(proj_thunks(b + 1))
                if b + 2 < NB:
                    filler_hi.append(lambda b=b: emit_xs_dma(b + 2))
                if b >= 1:
                    filler_lo.extend(outproj_thunks(b - 1))
                attention(b)
                drain_hi()
            filler_lo.extend(outproj_thunks(NB - 1))
            while filler_lo:
                filler_lo.pop(0)()

    nc.compile()
    return nc


def kernel(x, Wq, Wk, Wv, Wo, bo):
    x = np.asarray(x, dtype=np.float32)
    Wq = np.asarray(Wq, dtype=np.float32)
    Wk = np.asarray(Wk, dtype=np.float32)
    Wv = np.asarray(Wv, dtype=np.float32)
    Wo = np.asarray(Wo, dtype=np.float32)
    bo = np.asarray(bo, dtype=np.float32)

    if "nc" not in _CACHED:
        _CACHED["nc"] = _build()
    nc = _CACHED["nc"]

    bf = ml_dtypes.bfloat16
    in_maps = []
    for c in range(8):
        b, t = c // 4, c % 4
        xTc = np.ascontiguousarray(x[b].T).astype(bf)
        # q slab g holds [kv-head 2t head g | kv-head 2t+1 head g]
        qcols = []
        for g in range(4):
            for kvl in range(KVPC):
                h = (2 * t + kvl) * G + g
                qcols.append(Wq[:, h * DH:(h + 1) * DH])
        wq_c = np.ascontiguousarray(np.concatenate(qcols, axis=1)).astype(bf)
        wk_c = np.ascontiguousarray(Wk[:, t * 128:(t + 1) * 128]).astype(bf)
        wv_c = np.ascontiguousarray(Wv[:, t * 128:(t + 1) * 128]).astype(bf)
        # Wo rows must follow the ctxT head-pair layout: slab g holds
        # [head (kv 2t, g) | head (kv 2t+1, g)]
        wrows = []
        for g in range(4):
            for kvl in range(KVPC):
                h = (2 * t + kvl) * G + g
                wrows.append(Wo[h * DH:(h + 1) * DH, :])
        wo_c = np.ascontiguousarray(np.concatenate(wrows, axis=0)).astype(bf)
        in_maps.append({"xT": xTc, "Wq": wq_c, "Wk": wk_c, "Wv": wv_c,
                        "Wo": wo_c})

    trace = bool(int(os.environ.get("GQA_TRACE", "0")))
    kwargs = {}
    if trace:
        import tempfile
        td = os.environ.get("GQA_TRACE_DIR") or tempfile.mkdtemp(prefix="gqa_")
        kwargs = dict(trace=True, tmpdir=td)
    res = run_bass_kernel_spmd(nc, in_maps, list(range(8)), **kwargs)
    _CACHED["last_result"] = res

    out = np.empty((B, N, D), dtype=np.float32)
    for b in range(B):
        acc = res.results[4 * b]["out"].astype(np.float32)
        for t in range(1, 4):
            acc = acc + res.results[4 * b + t]["out"]
        out[b] = acc + bo[None, :]
    return out


# revision 15
# speedup vs baseline: 1.5471x; 1.2624x over previous
"""GQA forward (B=2,N=2048,D=2048,H=32,KV=8,DH=64, causal) on 8 trn2 cores.

Sharding: 2-way data parallel over batch x 4-way tensor parallel over heads
(each core: 8 q-heads = 2 kv-heads, keeping group structure). Row-parallel
out-proj; the all-reduce over the 4 TP shards (+ bias) happens on host at
gather time.

v2 design (vs baseline three serial phases):
  - all matmul operands bf16 (fp32 PSUM accumulation) -> FWL weight loads,
    half DMA/SBUF traffic, no fp32r narrow-moving penalty.
  - one fused pipeline: projection of q-block b+1 and out-projection of
    q-block b-1 are interleaved as PE filler between the attention matmuls
    of q-block b, so the tensor engine never idles long enough for the HAM
    clock gate to re-throttle to 1.2 GHz (the baseline spent 389us at half
    clock during attention).
  - scores for the 2 kv-heads of a head-pair run concurrently in PE row
    groups (K=64 contractions at base partitions 0 / 64).
  - causal mask applied by accumulating an identity-matmul of a -30000
    constant onto the diagonal score blocks (no DVE in the exp->ctx path).
  - exp batched: one ACT instruction per [128, 1024] PSUM span (both heads
    of a pair for one key block).
  - V projected directly in [tokens, dh] orientation with xs chunks as the
    stationary operand (no PE transposes).
"""
import os
import sys

import numpy as np

if "/opt/trn_rl_repo" not in sys.path:
    sys.path.insert(0, "/opt/trn_rl_repo")

import ml_dtypes

import concourse.bacc as bacc
import concourse.tile as tile
from concourse import mybir
from concourse.bass_utils import run_bass_kernel_spmd
from concourse.masks import make_identity

F32 = mybir.dt.float32
F32R = mybir.dt.float32r
BF16 = mybir.dt.bfloat16
EXP = mybir.ActivationFunctionType.Exp
LN = mybir.ActivationFunctionType.Ln

B, N, D = 2, 2048, 2048
H, KV, DH = 32, 8, 64
G = H // KV                      # 4 q-heads per kv head
HPC, KVPC = 8, 2                 # heads / kv-heads per core
DQ = HPC * DH                    # 512 per-core q projection width
NBW = 512                        # q-block width
NB = N // NBW                    # 4 q-blocks
DC = D // 128                    # 16 contraction chunks
NEG = -30000.0                   # causal mask additive constant

_CACHED = {}


def _build():
    nc = bacc.Bacc("TRN2", target_bir_lowering=False, debug=False,
                   num_devices=8)

    # Both Exp and Ln are used (softmax exp; 1/x as exp(-ln x)). The table
    # picker takes the first set containing each function, which would
    # alternate exp_and_others <-> natural_log at ~2.7us per switch. Trim
    # the cached table map (values only -- keys/order define act_func_set
    # ids and must stay) so the only set advertising Exp/Ln is the combined
    # one; it is then loaded exactly once.
    from concourse.hw_specs import get_activation_tables
    tabs = get_activation_tables(nc.m.arch)
    comb = tabs.get("natural_log_exp_and_others")
    if comb and EXP in comb and LN in comb:
        for name, fns in tabs.items():
            if name != "natural_log_exp_and_others":
                fns.discard(EXP)
                fns.discard(LN)

    xT = nc.dram_tensor("xT", [D, N], BF16, kind="ExternalInput")
    Wq = nc.dram_tensor("Wq", [D, DQ], BF16, kind="ExternalInput")
    Wk = nc.dram_tensor("Wk", [D, KVPC * DH], BF16, kind="ExternalInput")
    Wv = nc.dram_tensor("Wv", [D, KVPC * DH], BF16, kind="ExternalInput")
    Wo = nc.dram_tensor("Wo", [DQ, D], BF16, kind="ExternalInput")
    OUT = nc.dram_tensor("out", [N, D], F32, kind="ExternalOutput")

    scale = 1.0 / np.sqrt(DH)

    with tile.TileContext(nc) as tc:
        with (
            tc.tile_pool(name="persist", bufs=1) as pp,
            tc.tile_pool(name="wq", bufs=16) as wqp,
            tc.tile_pool(name="wkv", bufs=32) as wkvp,
            tc.tile_pool(name="wo", bufs=16) as wop,
            tc.tile_pool(name="xs", bufs=16) as xsp,
            tc.tile_pool(name="psb", bufs=4) as psbp,
            tc.tile_pool(name="outs", bufs=2) as outp,
            tc.tile_pool(name="small", bufs=2) as smp,
            tc.tile_pool(name="sc_ps", bufs=2, space="PSUM") as scp,
            tc.tile_pool(name="c_ps", bufs=2, space="PSUM") as cpp,
            tc.tile_pool(name="pj_ps", bufs=1, space="PSUM") as pjp,
            tc.tile_pool(name="ms_ps", bufs=1, space="PSUM") as msp,
        ):
            # ---- persistent sbuf state ----
            identf = pp.tile([128, 128], F32, tag="identf")
            make_identity(nc, identf[:])
            ident = pp.tile([128, 128], BF16, tag="ident")
            nc.vector.tensor_copy(ident[:], identf[:])

            # additive causal mask for a 128x128 diagonal block:
            # mask[k, j] = 0 if j >= k else NEG (local query j, local key k)
            mknf = pp.tile([128, 128], F32, tag="mknf")
            nc.gpsimd.memset(mknf[:], 0.0)
            nc.gpsimd.affine_select(
                out=mknf[:], in_=mknf[:],
                compare_op=mybir.AluOpType.is_ge,
                fill=NEG, base=0,
                pattern=[[1, 128]],
                channel_multiplier=-1,
            )
            maskneg = pp.tile([128, 128], BF16, tag="maskneg")
            nc.vector.tensor_copy(maskneg[:], mknf[:])

            onesf = pp.tile([128, 64], F32, tag="onesf")
            nc.vector.memset(onesf[:], 1.0)
            ones_b = pp.tile([128, 64], BF16, tag="onesb")
            nc.vector.tensor_copy(ones_b[:], onesf[:])
            ones_r = pp.tile([128, 64], F32R, tag="onesr")
            nc.vector.tensor_copy(ones_r[:], onesf[:])

            qt = [pp.tile([128, N], BF16, tag=f"qt{g}", name=f"qt{g}")
                  for g in range(4)]
            kt = pp.tile([128, N], BF16, tag="kt")
            # va[m]: [ A_dh(0:64) | onesA(64) | B_dh(65:129) | onesB(129) ]
            va = [pp.tile([128, 2 * (DH + 1)], BF16, tag=f"va{m}",
                          name=f"va{m}") for m in range(N // 128)]
            for m in range(N // 128):
                nc.vector.tensor_copy(va[m][:, DH:DH + 1], ones_b[:, 0:1])
                nc.vector.tensor_copy(va[m][:, 2 * DH + 1:2 * DH + 2],
                                      ones_b[:, 0:1])
            ctxT = [pp.tile([128, N], BF16, tag=f"ct{g}", name=f"ct{g}")
                    for g in range(4)]

            # ---- weights ----
            wq_sb, wk_sb, wv_sb = [], [], []
            for dc in range(DC):
                t = wqp.tile([128, DQ], BF16, tag="w")
                nc.scalar.dma_start(out=t[:], in_=Wq[dc * 128:(dc + 1) * 128, :])
                wq_sb.append(t)
            for dc in range(DC):
                t = wkvp.tile([128, KVPC * DH], BF16, tag="wk")
                nc.scalar.dma_start(out=t[:], in_=Wk[dc * 128:(dc + 1) * 128, :])
                wk_sb.append(t)
            for dc in range(DC):
                t = wkvp.tile([128, KVPC * DH], BF16, tag="wv")
                nc.scalar.dma_start(out=t[:], in_=Wv[dc * 128:(dc + 1) * 128, :])
                wv_sb.append(t)
            # wo rides the idle gpsimd queue: it is not needed until the
            # first out-projection (stage 1), and on sync it would delay
            # the xs streams.
            wo_sb = {}
            for j in range(4):
                for ob in range(4):
                    t = wop.tile([128, NBW], BF16, tag="wo")
                    nc.gpsimd.dma_start(
                        out=t[:],
                        in_=Wo[j * 128:(j + 1) * 128, ob * NBW:(ob + 1) * NBW])
                    wo_sb[(j, ob)] = t

            # ---- filler machinery ----------------------------------------
            # Thunks emitting PE-heavy work between attention ops so the
            # tensor engine never starves while ACT runs exp. hi = next
            # block's projections (must drain before that block's attention);
            # lo = previous block's out-projection (can carry over stages).
            filler_hi = []
            filler_lo = []

            def emit_filler(n):
                for _ in range(n):
                    if filler_hi:
                        filler_hi.pop(0)()
                    elif filler_lo:
                        filler_lo.pop(0)()
                    else:
                        break

            def drain_hi():
                while filler_hi:
                    filler_hi.pop(0)()

            xs_tiles = {}          # b -> list of 16 sbuf tiles

            def emit_xs_dma(b):
                ts = []
                for dc in range(DC):
                    t = xsp.tile([128, NBW], BF16, tag="xs", name="xs")
                    nc.sync.dma_start(
                        out=t[:],
                        in_=xT[dc * 128:(dc + 1) * 128,
                               b * NBW:(b + 1) * NBW])
                    ts.append(t)
                xs_tiles[b] = ts

            def proj_thunks(b):
                """Projection of q-block b: 6 bank-passes over resident xs,
                each pass split into 4-dc chunks (~850ns PE) for fine
                interleaving."""
                xs = xs_tiles[b]
                thunks = []

                def qk_chunk(g, c0, cell):
                    # g in 0..3 -> q slab g ; g == 4 -> k
                    def mk():
                        if c0 == 0:
                            cell.append(
                                pjp.tile([128, NBW], F32, tag="pj", name="pj"))
                        ps = cell[0]
                        for dc in range(c0, c0 + 4):
                            stat = (wq_sb[dc][:, g * 128:(g + 1) * 128]
                                    if g < 4 else wk_sb[dc][:])
                            nc.tensor.matmul(ps[:], stat, xs[dc][:],
                                             start=(dc == 0),
                                             stop=(dc == DC - 1))
                        if c0 + 4 == DC:
                            dst = qt[g] if g < 4 else kt
                            nc.vector.tensor_copy(
                                dst[:, b * NBW:(b + 1) * NBW], ps[:])
                    return mk

                def v_chunk(c0, cell):
                    def mk():
                        if c0 == 0:
                            cell.append(
                                pjp.tile([128, NBW], F32, tag="pj", name="pj"))
                        ps = cell[0]
                        for dc in range(c0, c0 + 4):
                            for i in range(4):
                                nc.tensor.matmul(
                                    ps[:, i * 128:(i + 1) * 128],
                                    xs[dc][:, i * 128:(i + 1) * 128],
                                    wv_sb[dc][:],
                                    start=(dc == 0 and i == 0),
                                    stop=(dc == DC - 1 and i == 3),
                                    skip_group_check=True)
                        if c0 + 4 == DC:
                            for i in range(4):
                                m = b * 4 + i
                                nc.vector.tensor_copy(
                                    va[m][:, 0:DH],
                                    ps[:, i * 128:i * 128 + 64])
                                nc.vector.tensor_copy(
                                    va[m][:, DH + 1:2 * DH + 1],
                                    ps[:, i * 128 + 64:i * 128 + 128])
                    return mk

                for g in range(5):
                    cell = []
                    for c0 in range(0, DC, 4):
                        thunks.append(qk_chunk(g, c0, cell))
                cell = []
                for c0 in range(0, DC, 4):
                    thunks.append(v_chunk(c0, cell))
                return thunks

            def outproj_thunks(b):
                """Out-projection of q-block b (4 row tiles)."""
                thunks = []

                def ob_unit(nt, ob, osb):
                    def mk():
                        ops = msp.tile([128, NBW], F32, tag="ms", name="ops")
                        for j in range(4):
                            nc.tensor.matmul(
                                ops[:],
                                ctxT[j][:, nt * 128:(nt + 1) * 128],
                                wo_sb[(j, ob)][:],
                                start=(j == 0), stop=(j == 3))
                        nc.vector.tensor_copy(
                            osb[0][:, ob * NBW:(ob + 1) * NBW], ops[:])
                    return mk

                def out_dma(nt, osb):
                    def mk():
                        nc.gpsimd.dma_start(
                            out=OUT[nt * 128:(nt + 1) * 128, :], in_=osb[0][:])
                    return mk

                for i in range(4):
                    nt = b * 4 + i
                    osb = []

                    def alloc(osb=osb):
                        osb.append(outp.tile([128, D], F32, tag="osb",
                                             name="osb"))
                    thunks.append(alloc)
                    for ob in range(4):
                        thunks.append(ob_unit(nt, ob, osb))
                    thunks.append(out_dma(nt, osb))
                return thunks

            # ---- norm: ctx / softmax-denominator, into ctxT ---------------
            def emit_pair_norm(c_a, c_b, g, q0):
                # Critical prefix: 4 plain copies move the ctx rows and the
                # denominator rows (psum partition 64, from the ones column
                # of va) out of PSUM so the two c banks free immediately.
                # Everything after runs async: broadcast both denominators
                # into one psum bank (col groups 0/64), 1/x as exp(-ln x)
                # on ACT (same table set as the attention exp; DVE
                # reciprocal costs ~3.3us per call), then normalize. Head
                # B's rows reach partitions 64:128 via a sbuf-to-sbuf DMA
                # (engines cannot shift partitions).
                lrowA = smp.tile([65, NBW], F32R, tag="lrA", name="lrA")
                nc.vector.tensor_copy(lrowA[DH:DH + 1, :], c_a[DH:DH + 1, :])
                lrowB = smp.tile([65, NBW], F32R, tag="lrB", name="lrB")
                nc.vector.tensor_copy(lrowB[DH:DH + 1, :], c_b[DH:DH + 1, :])
                cuA = smp.tile([DH, NBW], F32, tag="cuA", name="cuA")
                nc.vector.tensor_copy(cuA[:], c_a[0:DH, :])
                cuB = smp.tile([DH, NBW], F32, tag="cuB", name="cuB")
                nc.vector.tensor_copy(cuB[:], c_b[0:DH, :])
                for lrow, cu, par in ((lrowA, cuA, 0), (lrowB, cuB, 1)):
                    rb_ps = msp.tile([DH, NBW], F32, tag="ms", name="rbps")
                    nc.tensor.matmul(rb_ps[:], ones_r[DH:DH + 1, 0:DH],
                                     lrow[DH:DH + 1, :], start=True, stop=True)
                    lg = smp.tile([DH, NBW], F32, tag=f"lg{par}", name="lg")
                    nc.scalar.activation(lg[:], rb_ps[:], LN)
                    rb = smp.tile([DH, NBW], F32, tag=f"rb{par}", name="rb")
                    nc.scalar.activation(rb[:], lg[:], EXP, scale=-1.0)
                    if par == 0:
                        nc.vector.tensor_mul(ctxT[g][0:DH, q0:q0 + NBW],
                                             cu[:], rb[:])
                    else:
                        tmp = smp.tile([DH, NBW], BF16, tag="ctmp",
                                       name="ctmp")
                        nc.vector.tensor_mul(tmp[:], cu[:], rb[:])
                        nc.sync.dma_start(
                            out=ctxT[g][DH:2 * DH, q0:q0 + NBW], in_=tmp[:])

            # ---- attention for one q-block, with filler interleave --------
            def attention(b):
                q0 = b * NBW
                n_kb = 4 * b + 4
                for g in range(4):
                    c_a = cpp.tile([DH + 1, NBW], F32, tag="c", name="ca")
                    c_b = cpp.tile([DH + 1, NBW], F32, tag="c", name="cb")
                    pend = None
                    for kb in range(n_kb):
                        m0 = kb * 128
                        diag = kb >= 4 * b
                        off = max(0, m0 - q0)
                        T = scp.tile([128, 2 * NBW], F32, tag="sc", name="T")
                        # scores: kv0 rows 0:64 / kv1 rows 64:128 run as
                        # concurrent PE row-groups. Head A only needs cols
                        # >= off (earlier cols belong to fully-masked
                        # queries); head B keeps full width so the exp span
                        # [off : 2*NBW] reads initialized psum only.
                        nc.tensor.matmul(T[:, off:NBW],
                                         kt[0:64, m0:m0 + 128],
                                         qt[g][0:64, q0 + off:q0 + NBW],
                                         start=True, stop=not diag,
                                         skip_group_check=True)
                        nc.tensor.matmul(T[:, NBW:2 * NBW],
                                         kt[64:128, m0:m0 + 128],
                                         qt[g][64:128, q0:q0 + NBW],
                                         start=True, stop=not diag,
                                         skip_group_check=True)
                        if diag:
                            nc.tensor.matmul(T[:, off:off + 128],
                                             ident[:], maskneg[:],
                                             start=False, stop=True,
                                             skip_group_check=True)
                            nc.tensor.matmul(T[:, NBW + off:NBW + off + 128],
                                             ident[:], maskneg[:],
                                             start=False, stop=True,
                                             skip_group_check=True)
                        p = psbp.tile([128, 2 * NBW], BF16, tag="p", name="p")
                        nc.scalar.activation(p[:, off:2 * NBW],
                                             T[:, off:2 * NBW], EXP,
                                             scale=float(scale))
                        if pend is not None:
                            pend()
                        emit_filler(1)

                        def _ctx(kb=kb, off=off, p=p, c_a=c_a, c_b=c_b):
                            nc.tensor.matmul(
                                c_a[:, off:NBW],
                                va[kb][:, 0:DH + 1],
                                p[:, off:NBW],
                                start=(kb == 0), stop=(kb == n_kb - 1),
                                skip_group_check=True)
                            nc.tensor.matmul(
                                c_b[:, off:NBW],
                                va[kb][:, DH + 1:2 * DH + 2],
                                p[:, NBW + off:2 * NBW],
                                start=(kb == 0), stop=(kb == n_kb - 1),
                                skip_group_check=True)
                        pend = _ctx
                    pend()
                    emit_pair_norm(c_a, c_b, g, q0)
                    emit_filler(2)

            # ---- main schedule -------------------------------------------
            emit_xs_dma(0)
            for th in proj_thunks(0):
                th()
            emit_xs_dma(1)
            for b in range(NB):
                if b + 1 < NB:
                    filler_hi.extend(proj_thunks(b + 1))
                if b + 2 < NB:
                    filler_hi.append(lambda b=b: emit_xs_dma(b + 2))
                if b >= 1:
                    filler_lo.extend(outproj_thunks(b - 1))
                attention(b)
                drain_hi()
            filler_lo.extend(outproj_thunks(NB - 1))
            while filler_lo:
                filler_lo.pop(0)()

    nc.compile()
    return nc


def kernel(x, Wq, Wk, Wv, Wo, bo):
    x = np.asarray(x, dtype=np.float32)
    Wq = np.asarray(Wq, dtype=np.float32)
    Wk = np.asarray(Wk, dtype=np.float32)
    Wv = np.asarray(Wv, dtype=np.float32)
    Wo = np.asarray(Wo, dtype=np.float32)
    bo = np.asarray(bo, dtype=np.float32)

    if "nc" not in _CACHED:
        _CACHED["nc"] = _build()
    nc = _CACHED["nc"]

    bf = ml_dtypes.bfloat16
    in_maps = []
    for c in range(8):
        b, t = c // 4, c % 4
        xTc = np.ascontiguousarray(x[b].T).astype(bf)
        # q slab g holds [kv-head 2t head g | kv-head 2t+1 head g]
        qcols = []
        for g in range(4):
            for kvl in range(KVPC):
                h = (2 * t + kvl) * G + g
                qcols.append(Wq[:, h * DH:(h + 1) * DH])
        wq_c = np.ascontiguousarray(np.concatenate(qcols, axis=1)).astype(bf)
        wk_c = np.ascontiguousarray(Wk[:, t * 128:(t + 1) * 128]).astype(bf)
        wv_c = np.ascontiguousarray(Wv[:, t * 128:(t + 1) * 128]).astype(bf)
        # Wo rows must follow the ctxT head-pair layout: slab g holds
        # [head (kv 2t, g) | head (kv 2t+1, g)]
        wrows = []
        for g in range(4):
            for kvl in range(KVPC):
                h = (2 * t + kvl) * G + g
                wrows.append(Wo[h * DH:(h + 1) * DH, :])
        wo_c = np.ascontiguousarray(np.concatenate(wrows, axis=0)).astype(bf)
        in_maps.append({"xT": xTc, "Wq": wq_c, "Wk": wk_c, "Wv": wv_c,
                        "Wo": wo_c})

    trace = bool(int(os.environ.get("GQA_TRACE", "0")))
    kwargs = {}
    if trace:
        import tempfile
        td = os.environ.get("GQA_TRACE_DIR") or tempfile.mkdtemp(prefix="gqa_")
        kwargs = dict(trace=True, tmpdir=td)
    res = run_bass_kernel_spmd(nc, in_maps, list(range(8)), **kwargs)
    _CACHED["last_result"] = res

    out = np.empty((B, N, D), dtype=np.float32)
    for b in range(B):
        acc = res.results[4 * b]["out"].astype(np.float32)
        for t in range(1, 4):
            acc = acc + res.results[4 * b + t]["out"]
        out[b] = acc + bo[None, :]
    return out
